# revision 72
# baseline (speedup 1.0000x reference)
"""Bag self-attention kernel for TRN2, data-parallel over the bag dim (8 cores).

Per core (one bag, x: [N=2048, L=1280], H=160):
  q = x@Wq.T + bq ; k = x@Wk.T (bk cancels) ; v = x@Wv.T
  S = q@k.T ; P = softmax(S) ; out = P@v + (x + bv)      (gamma = 1)

Mixed-precision split, driven by softmax sensitivity (logit noise at
near-tie rows is amplified by the value spread, so the q/k path needs
~FP22 while v and the attention weights tolerate fp8 pairs):

  - q/k projections and the energies S run in float32r (FP22 grade).
  - v projection and P@v run as fp8-e4m3 DoubleRow matmuls (2 k-tiles of
    128 per instruction, 0.5 PE cycles per output row). Operands are
    hi/lo fp8 splits (value = hi + lo); 3-term products
    A@B ~= Ah@Bh + Al@Bh + Ah@Bl give ~2^-9 relative error. Wv is
    pre-scaled by 64 on host so no fp8 entry is subnormal; the f32 PSUM
    result is descaled by 1/64 when re-quantized.
  - Softmax without transposes: pass-1 computes approximate S in [i,j]
    layout (single fp8 term from fp8 copies of q/k), DVE row-max gives
    m_i; the shift c_i = -(m_i - 1.5) is transposed into row form by
    tiny PE transposes and stored as an f32 augment row (partition 32)
    of the packed q1 tile, with a matching ones row in the k1 tile.
    Pass-2 computes S - c_i in [j,i] layout in f32r; ACT exp writes fp8
    E directly (E_top in ~[0.8, 25], inside e4m3's 240 max). Z comes
    from a ones-column of v; out = (E@v)/Z + (x + bv).
  - P@v is 2-term (Eh@vh + Eh@vl): v is kept exact via its fp8 pair, E
    carries the e4m3 quantization noise (renormalized by Z from the same
    quantized E, so one-hot rows cancel exactly).

Scheduling: the q/k projection stream (f32 x, DMA-heavy, PE-light) is
merged with the v-projection stream (fp8, PE-heavy) so the serialized
DMA pipe and the PE stay busy simultaneously. All input loads ride the
ACT HWDGE queue with posts staggered through the instruction stream
(pipe executes transfers in post order); output stores alternate the
SP/ACT queues. Biases are applied as per-partition ACT bias on the
PSUM->SBUF copies instead of extra bias matmul passes.
"""

import contextlib

import numpy as np
import ml_dtypes

import concourse.mybir as mybir
import concourse.tile as tile
from concourse import bacc
from concourse.bass_utils import run_bass_kernel_spmd

B, N, L, H = 8, 2048, 1280, 160
f32 = mybir.dt.float32
f32r = mybir.dt.float32r
bf16 = mybir.dt.bfloat16
fp8 = mybir.dt.float8e4
FP8 = ml_dtypes.float8_e4m3
DR = mybir.MatmulPerfMode.DoubleRow
Exp = mybir.ActivationFunctionType.Exp
Copy = mybir.ActivationFunctionType.Copy
Ident = mybir.ActivationFunctionType.Identity
ADD = mybir.AluOpType.add
SUB = mybir.AluOpType.subtract
MULT = mybir.AluOpType.mult
MAX = mybir.AluOpType.max

NL = L // 128          # 10 contraction k-tiles
NP = NL // 2           # 5 DoubleRow pairs
H0, H1 = 128, H - 128  # q/k head split 128 + 32
WS = 64.0              # host Wv scale (keeps fp8 Wv out of subnormals)
CM = 1.5               # row-max shift margin
NJ = N // 128          # 16 token chunks
NI4 = N // 512         # 4 i-macro chunks
MCH = [(1024, 1282), (0, 512), (512, 1024)]   # P@v m-chunks, Z-chunk first
ZC = 1280              # ones column (Z) position in v
VW = 1312              # v tile free width


def _build():
    nc = bacc.Bacc()
    dp = nc.declare_dram_parameter
    xf_d = dp("xf", [128, NL * N], f32r, isOutput=False)      # f32 xT [p,c,n]
    xh_d = dp("xh", [128, NL * N], fp8, isOutput=False)
    xl_d = dp("xl", [128, NL * N], fp8, isOutput=False)
    wq_d = dp("wq", [128, NL * H0], f32r, isOutput=False)
    wk_d = dp("wk", [128, NL * H0], f32r, isOutput=False)
    w1_d = dp("w1", [128, NL * 2 * H1], f32r, isOutput=False)  # q1|k1 packed
    wvh_d = dp("wvh", [128, NL * L], fp8, isOutput=False)
    wvl_d = dp("wvl", [128, NL * L], fp8, isOutput=False)
    bq0_d = dp("bq0", [128, 1], f32, isOutput=False)
    b1_d = dp("b1", [H1, 1], f32, isOutput=False)
    xr_d = dp("xresid", [N, L], f32, isOutput=False)
    id_d = dp("ident", [128, 128], f32, isOutput=False)
    z32_d = dp("zeros32", [32, N], f32r, isOutput=False)
    k1g_d = dp("k1aug", [32, N], f32r, isOutput=False)
    out_d = dp("out", [N, L], f32, isOutput=True)

    with tile.TileContext(nc) as tc:
        with (
            tc.tile_pool(name="const", bufs=1) as constp,
            tc.tile_pool(name="qk", bufs=1) as qkp,
            tc.tile_pool(name="vt", bufs=1) as vtp,
        ):
            es5 = contextlib.ExitStack()
            ep = es5.enter_context(tc.tile_pool(name="ep", bufs=2))
            es = contextlib.ExitStack()
            xtp = es.enter_context(tc.tile_pool(name="xt", bufs=1))
            wvp = es.enter_context(tc.tile_pool(name="wv", bufs=1, side="right"))
            wp = es.enter_context(tc.tile_pool(name="wp", bufs=1, side="right"))
            # ---- resident tiles (xt/wv/wp pools close before phase 5)
            xh = [xtp.tile([128, NL, 512], fp8, tag=f"xh{g}", name=f"xh{g}")
                  for g in range(4)]
            xl = [xtp.tile([128, NL, 512], fp8, tag=f"xl{g}", name=f"xl{g}")
                  for g in range(4)]
            wvh = [wvp.tile([128, NL, 512], fp8, tag="wvh", bufs=3,
                            name=f"wvh{mc}") for mc in range(3)]
            wvl = [wvp.tile([128, NL, 512], fp8, tag="wvl", bufs=3,
                            name=f"wvl{mc}") for mc in range(3)]
            wq = wp.tile([128, NL, H0], f32r, tag="wq")
            wka = wp.tile([128, 5, H0], f32r, tag="wka")
            wkb = wp.tile([128, 5, H0], f32r, tag="wkb")
            w1 = wp.tile([128, NL, 2 * H1], f32r, tag="w1")
            bq0 = wp.tile([128, 1], f32, tag="bq0")
            b1 = wp.tile([H1, 1], f32, tag="b1")
            ident = constp.tile([128, 128], f32, tag="ident")
            # f32r q/k: q0/k0 [128, N]; packed 32-head chunk + augments in
            # [64, N] tiles (q1a: rows 0..31 = q1, row 32 = -c_i, 33.. = 0;
            # k1a: rows 0..31 = k1, row 32 = ones, 33.. = 0)
            q0f = qkp.tile([128, N], f32r, tag="q0f")
            k0f = qkp.tile([128, N], f32r, tag="k0f")
            q1a = qkp.tile([64, N], f32r, tag="q1a")
            k1a = qkp.tile([64, N], f32r, tag="k1a")
            # fp8 copies of q/k for the pass-1 max estimate
            qh = qkp.tile([128, 2, N], fp8, tag="qh")
            kh = qkp.tile([128, 2, N], fp8, tag="kh")
            vh = [vtp.tile([128, 2, VW], fp8, tag=f"vh{jp}", name=f"vh{jp}")
                  for jp in range(8)]
            vl = [vtp.tile([128, 2, VW], fp8, tag=f"vl{jp}", name=f"vl{jp}")
                  for jp in range(8)]
            mall = constp.tile([128, 16], f32, tag="mall")
            mall4 = constp.tile([128, 64], f32, tag="mall4")

            xf_r = xf_d.rearrange("p (c n) -> p c n", c=NL)
            xh_r = xh_d.rearrange("p (c n) -> p c n", c=NL)
            xl_r = xl_d.rearrange("p (c n) -> p c n", c=NL)
            wvh_r = wvh_d.rearrange("p (c m) -> p c m", c=NL)
            wvl_r = wvl_d.rearrange("p (c m) -> p c m", c=NL)

            # memsets on Pool (no PE dependency)
            for t in (qh, kh):
                for p0 in (32, 64, 96):
                    nc.gpsimd.memset(t[p0:p0 + 32, 1, :], 0.0)
            for jp in range(8):
                nc.gpsimd.memset(vh[jp][:, :, ZC:VW], 0.0)
                nc.gpsimd.memset(vl[jp][:, :, ZC:VW], 0.0)
                nc.gpsimd.memset(vh[jp][:, :, ZC:ZC + 1], 1.0)

            # ---- fp8 x / Wv splits ride the ACT queue
            def xg_load(g, q=nc.scalar):
                csl = slice(g * 512, (g + 1) * 512)
                q.dma_start(out=xh[g], in_=xh_r[:, :, csl])
                q.dma_start(out=xl[g], in_=xl_r[:, :, csl])

            def wv_load(mc, q=nc.scalar):
                mlo2 = mc * 512
                mhi2 = min(mlo2 + 512, L)
                q.dma_start(out=wvh[mc][:, :, 0:mhi2 - mlo2],
                            in_=wvh_r[:, :, mlo2:mhi2])
                q.dma_start(out=wvl[mc][:, :, 0:mhi2 - mlo2],
                            in_=wvl_r[:, :, mlo2:mhi2])


            def acc3(ps, stat_h, stat_l, mov_h, mov_l):
                """fp8 DoubleRow 3-term product into ps."""
                for t in range(NP):
                    nc.tensor.matmul(ps, stat_h[:, 2 * t:2 * t + 2, :],
                                     mov_h[:, 2 * t:2 * t + 2, :],
                                     start=(t == 0), stop=False, perf_mode=DR)
                for t in range(NP):
                    nc.tensor.matmul(ps, stat_h[:, 2 * t:2 * t + 2, :],
                                     mov_l[:, 2 * t:2 * t + 2, :],
                                     start=False, stop=False, perf_mode=DR)
                for t in range(NP):
                    nc.tensor.matmul(ps, stat_l[:, 2 * t:2 * t + 2, :],
                                     mov_h[:, 2 * t:2 * t + 2, :],
                                     start=False, stop=(t == NP - 1),
                                     perf_mode=DR)

            ph5 = {}

            def s2_unit(i4, j, eh):
                isl = slice(i4 * 512, (i4 + 1) * 512)
                jsl = slice(j * 128, (j + 1) * 128)
                jp, half = j // 2, j % 2
                ps = ph5["s2ps"].tile([128, 512], f32, tag="s2",
                                      name=f"s2_{i4}_{j}")
                nc.tensor.matmul(ps, k0f[:, jsl], q0f[:, isl],
                                 start=True, stop=False)
                nc.tensor.matmul(ps, k1a[:, jsl], q1a[:, isl],
                                 start=False, stop=True)
                nc.scalar.activation(eh[jp][:, half, :], ps, Exp)

            def mk_e(i4):
                return [ep.tile([128, 2, 512], fp8, tag=f"eh{jp}",
                                name=f"eh{i4}_{jp}") for jp in range(8)]

            # ---- phases 1-4 merged. H-units (f32r q/k projection over 256
            # token cols) stream f32 x on the SP queue; V-units (fp8 v-proj
            # for one (tok, mc) chunk) consume the ACT-queue fp8 stream; the
            # pass-1 S~ units + row-max reduces + c augment rows follow the
            # last H-unit, and the S2(i4=0) exps close the phase.
            with (
                tc.tile_pool(name="vps", bufs=3, space="PSUM") as vps,
                tc.tile_pool(name="v32p", bufs=2) as v32p,
            ):
                def v_unit(tok, mc, dve):
                    g, gt = tok // 4, tok % 4
                    jsl = slice(gt * 128, (gt + 1) * 128)
                    jp, half = tok // 2, tok % 2
                    mlo, mhi = ((0, 512), (512, 1024), (1024, 1280))[mc]
                    ps = vps.tile([128, 512], f32, tag="v",
                                  name=f"v{tok}_{mc}")
                    psv = ps[:, 0:mhi - mlo]
                    acc3(psv, xh[g][:, :, jsl], xl[g][:, :, jsl],
                         wvh[mc][:, :, 0:mhi - mlo], wvl[mc][:, :, 0:mhi - mlo])
                    nc.scalar.activation(vh[jp][:, half, mlo:mhi], psv,
                                         Copy, scale=1.0 / WS)
                    if dve:
                        nc.vector.scalar_tensor_tensor(
                            out=vl[jp][:, half, mlo:mhi], in0=psv,
                            scalar=1.0 / WS, in1=vh[jp][:, half, mlo:mhi],
                            op0=MULT, op1=SUB)
                    else:
                        v32 = v32p.tile([128, 512], f32, tag="v32",
                                        name=f"v32_{tok}_{mc}")
                        nc.scalar.activation(v32[:, 0:mhi - mlo], psv, Copy,
                                             scale=1.0 / WS)
                        nc.gpsimd.tensor_sub(vl[jp][:, half, mlo:mhi],
                                             v32[:, 0:mhi - mlo],
                                             vh[jp][:, half, mlo:mhi])

                with (
                    tc.tile_pool(name="qkps", bufs=1, space="PSUM") as qkps,
                    tc.tile_pool(name="xfp", bufs=2) as xfp,
                ):
                    xfs = {}

                    def xf_load(hg):
                        # two c-halves: the consuming H-units' first five
                        # matmuls (k-tiles 0-4) unlock on the first half
                        if hg < 8:
                            xfs[hg] = xfp.tile([128, NL, 256], f32r, tag="xf",
                                               name=f"xf{hg}")
                            csl = slice(hg * 256, (hg + 1) * 256)
                            nc.scalar.dma_start(out=xfs[hg][:, 0:5, :],
                                                in_=xf_r[:, 0:5, csl])
                            nc.scalar.dma_start(out=xfs[hg][:, 5:10, :],
                                                in_=xf_r[:, 5:10, csl])

                    # ---- phase-1-4 input loads ride the single ACT queue.
                    # The serial DMA pipe runs transfers in post order; posts
                    # are staggered through the ACT instruction stream so the
                    # pipe order tracks the need order without conservative
                    # sem waits on large post batches.
                    nc.scalar.dma_start(out=wka[:, 0:3, :],
                                        in_=wk_d[:, 0:3 * H0])
                    xfs[0] = xfp.tile([128, NL, 256], f32r, tag="xf",
                                      name="xf0")
                    nc.scalar.dma_start(out=xfs[0][:, 0:3, :],
                                        in_=xf_r[:, 0:3, 0:256])
                    nc.scalar.dma_start(out=wka[:, 3:5, :],
                                        in_=wk_d[:, 3 * H0:5 * H0])
                    nc.scalar.dma_start(out=xfs[0][:, 3:6, :],
                                        in_=xf_r[:, 3:6, 0:256])
                    nc.scalar.dma_start(out=wkb, in_=wk_d[:, 5 * H0:NL * H0])
                    nc.scalar.dma_start(out=xfs[0][:, 6:10, :],
                                        in_=xf_r[:, 6:10, 0:256])
                    nc.scalar.dma_start(out=wq[:, 0:5, :],
                                        in_=wq_d[:, 0:5 * H0])
                    nc.scalar.dma_start(out=wq[:, 5:10, :],
                                        in_=wq_d[:, 5 * H0:NL * H0])
                    xf_load(1)
                    nc.scalar.dma_start(out=w1, in_=w1_d[:, :])
                    nc.scalar.dma_start(out=bq0, in_=bq0_d[:, :])
                    nc.scalar.dma_start(out=b1, in_=b1_d[:, :])

                    def hk_unit(hg):
                        isl = slice(hg * 256, (hg + 1) * 256)
                        xf = xfs[hg]
                        wkc = lambda c: (wka[:, c, :] if c < 5
                                         else wkb[:, c - 5, :])
                        ps_k = qkps.tile([H0, 256], f32, tag="k", name=f"k{hg}")
                        for c in range(NL):
                            nc.tensor.matmul(ps_k, wkc(c), xf[:, c, :],
                                             start=(c == 0), stop=(c == NL - 1))
                        nc.scalar.activation(k0f[:, isl], ps_k, Copy)
                        nc.scalar.activation(kh[:, 0, isl], ps_k, Copy)

                    def hq_unit(hg):
                        isl = slice(hg * 256, (hg + 1) * 256)
                        xf = xfs[hg]
                        ps_q = qkps.tile([H0, 256], f32, tag="q", name=f"q{hg}")
                        for c in range(NL):
                            nc.tensor.matmul(ps_q, wq[:, c, :], xf[:, c, :],
                                             start=(c == 0), stop=(c == NL - 1))
                        nc.scalar.activation(q0f[:, isl], ps_q, Ident,
                                             bias=bq0)
                        nc.scalar.activation(qh[:, 0, isl], ps_q, Ident,
                                             bias=bq0)

                    def h1_unit(hg):
                        isl = slice(hg * 256, (hg + 1) * 256)
                        xf = xfs.pop(hg)
                        ps_1 = qkps.tile([2 * H1, 256], f32, tag="qk1",
                                         name=f"qk1{hg}")
                        for c in range(NL):
                            nc.tensor.matmul(ps_1, w1[:, c, :], xf[:, c, :],
                                             start=(c == 0), stop=(c == NL - 1))
                        nc.scalar.activation(q1a[0:32, isl], ps_1[0:H1, :],
                                             Ident, bias=b1)
                        nc.scalar.activation(qh[0:H1, 1, isl], ps_1[0:H1, :],
                                             Ident, bias=b1)
                        nc.scalar.activation(k1a[0:32, isl], ps_1[H1:2 * H1, :],
                                             Copy)
                        nc.scalar.activation(kh[0:H1, 1, isl],
                                             ps_1[H1:2 * H1, :], Copy)
                        xf_load(hg + 2)

                    def h_trio(hg):
                        hk_unit(hg)
                        hq_unit(hg)
                        h1_unit(hg)

                    # g0 loads interleaved by first-use inside acc3: the
                    # hi@hi term needs only wvh0+xh0, hi@lo adds wvl0, and
                    # lo@hi adds xl0 last
                    nc.scalar.dma_start(out=wvh[0], in_=wvh_r[:, :, 0:512])
                    nc.scalar.dma_start(out=xh[0], in_=xh_r[:, :, 0:512])
                    nc.scalar.dma_start(out=wvl[0], in_=wvl_r[:, :, 0:512])
                    nc.scalar.dma_start(out=xl[0], in_=xl_r[:, :, 0:512])
                    h_trio(0)
                    h_trio(1)
                    v_unit(0, 0, dve=True)
                    v_unit(1, 0, dve=True)
                    h_trio(2)
                    wv_load(1)
                    v_unit(2, 0, dve=True)
                    v_unit(3, 0, dve=True)
                    h_trio(3)
                    xg_load(1)
                    v_unit(0, 1, dve=True)
                    v_unit(1, 1, dve=True)
                    h_trio(4)
                    wv_load(2)
                    v_unit(2, 1, dve=True)
                    v_unit(3, 1, dve=True)
                    h_trio(5)
                    xg_load(2)
                    v_unit(4, 0, dve=True)
                    v_unit(5, 0, dve=True)
                    h_trio(6)
                    xg_load(3)
                    v_unit(6, 0, dve=True)
                    v_unit(7, 0, dve=True)
                    h_trio(7)
                    nc.scalar.dma_start(out=ident, in_=id_d[:, :])
                    nc.scalar.dma_start(out=q1a[32:64, :], in_=z32_d[:, :])
                    nc.scalar.dma_start(out=k1a[32:64, :], in_=k1g_d[:, :])
                    v_unit(4, 1, dve=True)
                    v_unit(5, 1, dve=True)

                s1_long = ([(t, 0) for t in range(8, 16)]
                           + [(t, 1) for t in range(8, 16)])
                s1_short = [(t, 2) for t in range(16)]
                s1_short_at = {ic: 1 for ic in range(16)}
                s1_short_at[0] = 2
                s1_short_at[15] = 0
                tail_v = [(t, 1) for t in (6, 7)]

                cstack = contextlib.ExitStack()
                cps = cstack.enter_context(
                    tc.tile_pool(name="cps", bufs=1, space="PSUM"))
                with tc.tile_pool(name="s1ps", bufs=2, space="PSUM") as s1ps:
                    def c_rows(ic):
                        isl = slice(ic * 128, (ic + 1) * 128)
                        pt = cps.tile([1, 128], f32, tag="ct", name=f"ct{ic}")
                        nc.tensor.matmul(pt, mall[:, ic:ic + 1], ident,
                                         is_transpose=True)
                        nc.scalar.activation(q1a[32:33, isl], pt, Copy, bias=CM)

                    # pass-1 S~ + row-max: per ic, two [128, 1024] PSUM
                    # halves (2 DR matmuls + one DVE reduce each, chunk maxes
                    # into mall4, second-stage max into mall). bufs=2 means a
                    # half's buffer is recycled only after its reduce from the
                    # previous ic, which completed long ago, so the PE never
                    # waits on the DVE chain. V-units (gpsimd vl path, keeping
                    # the DVE clear for reduces) fill the PE.
                    ri = 0
                    for ic in range(16):
                        for jh in range(2):
                            s1t = s1ps.tile([128, 1024], f32, tag="s1",
                                            name=f"s1_{ic}_{jh}")
                            for jc in (2 * jh, 2 * jh + 1):
                                nc.tensor.matmul(
                                    s1t[:, (jc % 2) * 512:(jc % 2) * 512 + 512],
                                    qh[:, :, ic * 128:(ic + 1) * 128],
                                    kh[:, :, jc * 512:(jc + 1) * 512],
                                    start=True, stop=True, perf_mode=DR)
                            nc.vector.tensor_reduce(
                                mall4[:, 2 * ic + jh:2 * ic + jh + 1], s1t,
                                axis=mybir.AxisListType.X, op=MAX)
                        nc.vector.tensor_reduce(
                            mall[:, ic:ic + 1], mall4[:, 2 * ic:2 * ic + 2],
                            axis=mybir.AxisListType.X, op=MAX, negate=True)
                        v_unit(*s1_long[ic], dve=False)
                        for _ in range(s1_short_at.get(ic, 0)):
                            v_unit(*s1_short[ri], dve=False)
                            ri += 1
                        if ic >= 2:
                            c_rows(ic - 2)

                # s1ps/cps banks are free now for the i4=0 S2 tail
                # tail: remaining v units + late c rows + the S2(i4=0) units
                # (s2(0,*) reads only q1a cols 0:512, i.e. c0-3, so c14/c15
                # can ride along here without stalling the PE). Shares the
                # phase-5 s2 PSUM pool so there is no pool-close barrier at
                # the phase boundary.
                e_cur = mk_e(0)
                s2ps = es5.enter_context(
                    tc.tile_pool(name="s2ps", bufs=4, space="PSUM",
                                 side="right"))
                ph5["s2ps"] = s2ps
                ti, s2j = 0, 0
                while ti < len(tail_v) or s2j < NJ:
                    if ti < len(tail_v):
                        v_unit(*tail_v[ti], dve=True)
                        ti += 1
                    if s2j == 2:
                        c_rows(14)
                    elif s2j == 4:
                        c_rows(15)
                    if s2j < NJ:
                        s2_unit(0, s2j, e_cur)
                        s2j += 1
                cstack.close()

            es.close()   # free x / wv / weight SBUF before attention
            # ---- phase 5: attention; S2(i4) interleaved with P@v(i4-1)
            with (
                tc.tile_pool(name="ops", bufs=1, space="PSUM") as ops,
                tc.tile_pool(name="stg", bufs=2) as stg,
            ):
                def pv_unit(i4, isub, mc, pso, eh, split_out=False,
                            mch=None, st_q=None):
                    i0 = i4 * 512 + isub * 128
                    esl = slice(isub * 128, (isub + 1) * 128)
                    mlo, mhi = mch if mch is not None else MCH[mc]
                    ps = ops.tile([128, 512], f32, tag=f"o{mc}",
                                  bufs=(2 if mc == 2 else 1),
                                  name=f"o{i4}_{isub}_{mc}_{mlo}")
                    ps = ps[:, 0:mhi - mlo]
                    for jp in range(8):
                        nc.tensor.matmul(
                            ps, eh[jp][:, :, esl], vh[jp][:, :, mlo:mhi],
                            start=(jp == 0), stop=False, perf_mode=DR)
                    for jp in range(8):
                        nc.tensor.matmul(
                            ps, eh[jp][:, :, esl], vl[jp][:, :, mlo:mhi],
                            start=False, stop=(jp == 7), perf_mode=DR)
                    if mc == 0:
                        recip = stg.tile([128, 1], f32, tag="recip",
                                         name=f"recip{i4}_{isub}")
                        pso["recip"] = recip
                        nc.vector.reciprocal(recip, ps[:, 256:257])
                        xr = stg.tile([128, L], f32, tag="xr",
                                      name=f"xr{i4}_{isub}")
                        pso["xr"] = xr
                        nc.sync.dma_start(out=xr, in_=xr_d[i0:i0 + 128, :])
                    mwid = min(mhi, L) - mlo
                    ot = stg.tile([128, 512], f32, tag=f"ot{mc}",
                                  name=f"ot{i4}_{isub}_{mc}_{mlo}")
                    halves = ((0, mwid // 2), (mwid // 2, mwid)) if split_out \
                        else ((0, mwid),)
                    for nh, (lo, hi) in enumerate(halves):
                        nc.vector.scalar_tensor_tensor(
                            out=ot[:, lo:hi], in0=ps[:, lo:hi],
                            scalar=pso["recip"],
                            in1=pso["xr"][:, mlo + lo:mlo + hi],
                            op0=MULT, op1=ADD)
                        q = st_q if st_q is not None else nc.sync
                        q.dma_start(
                            out=out_d[i0:i0 + 128, mlo + lo:mlo + hi],
                            in_=ot[:, lo:hi])

                for i4 in range(1, NI4 + 1):
                    e_prev, pso_prev = e_cur, {}
                    pv_units = [(isub, mc) for isub in range(4)
                                for mc in range(3)]
                    if i4 <= NI4 - 1:
                        e_cur = mk_e(i4)
                        if i4 == 1:
                            pv_at = {5: (0, 3), 9: (3, 6), 12: (6, 9),
                                     15: (9, 12)}
                        else:
                            pv_at = {j: (j - 2, j - 1) for j in range(2, 14)}
                        for j in range(NJ):
                            s2_unit(i4, j, e_cur)
                            if j in pv_at:
                                lo, hi = pv_at[j]
                                for isub, mc in pv_units[lo:hi]:
                                    pv_unit(i4 - 1, isub, mc, pso_prev,
                                            e_prev)
                    else:
                        for un, (isub, mc) in enumerate(pv_units[:-1]):
                            pv_unit(i4 - 1, isub, mc, pso_prev, e_prev,
                                    split_out=(un >= 9),
                                    st_q=(nc.scalar if un % 2 else nc.sync))
                        # last unit (isub 3, mc 2) in two m-halves so the
                        # first half's store chain overlaps the second
                        # half's matmuls
                        mlo, mhi = MCH[2]
                        mmid = (mlo + mhi) // 2
                        pv_unit(i4 - 1, 3, 2, pso_prev, e_prev,
                                mch=(mlo, mmid), st_q=nc.scalar)
                        pv_unit(i4 - 1, 3, 2, pso_prev, e_prev,
                                mch=(mmid, mhi), st_q=nc.sync)
            es5.close()

    nc.finalize()
    return nc


_NC = None


def _get_nc():
    global _NC
    if _NC is None:
        _NC = _build()
    return _NC


def _split8(a):
    hi = a.astype(FP8)
    lo = (a - hi.astype(np.float32)).astype(FP8)
    return hi, lo


def _wpackf(WT):
    """WT: [L, h] f32. Returns [128, NL*h] f32 with k-tile layout
    [p, c, h]."""
    Lh, h = WT.shape
    full = WT.reshape(NL, 128, h).transpose(1, 0, 2)
    return np.ascontiguousarray(full.reshape(128, NL * h))


def kernel(x, Wq, bq, Wk, bk, Wv, bv):
    x = np.asarray(x, np.float32)
    Wq = np.asarray(Wq, np.float32); bq = np.asarray(bq, np.float32)
    Wk = np.asarray(Wk, np.float32)
    Wv = np.asarray(Wv, np.float32); bv = np.asarray(bv, np.float32)

    WqT = Wq.T                    # [L, H]
    WkT = Wk.T
    wqf = _wpackf(np.ascontiguousarray(WqT[:, :H0]))
    wkf = _wpackf(np.ascontiguousarray(WkT[:, :H0]))
    w1c = np.concatenate([WqT[:, H0:], WkT[:, H0:]], axis=1)  # [L, 64]
    w1f = _wpackf(np.ascontiguousarray(w1c))
    WvTs = Wv.T * WS
    wvh_, wvl_ = _split8(
        WvTs.reshape(NL, 128, L).transpose(1, 0, 2))
    wvh = np.ascontiguousarray(wvh_.reshape(128, NL * L))
    wvl = np.ascontiguousarray(wvl_.reshape(128, NL * L))

    nc = _get_nc()
    ident = np.eye(128, dtype=np.float32)
    z32_h = np.zeros((32, N), np.float32)
    k1g_h = np.zeros((32, N), np.float32)
    k1g_h[0, :] = 1.0
    bq0_h = np.ascontiguousarray(bq[:H0, None])
    b1_h = np.ascontiguousarray(bq[H0:, None])
    in_maps = []
    for b in range(B):
        xT3 = np.ascontiguousarray(x[b].T).reshape(NL, 128, N).transpose(1, 0, 2)
        xh, xl = _split8(xT3)
        in_maps.append({
            "xf": np.ascontiguousarray(xT3.reshape(128, NL * N)),
            "xh": np.ascontiguousarray(xh.reshape(128, NL * N)),
            "xl": np.ascontiguousarray(xl.reshape(128, NL * N)),
            "wq": wqf, "wk": wkf, "w1": w1f, "wvh": wvh, "wvl": wvl,
            "bq0": bq0_h, "b1": b1_h,
            "xresid": x[b] + bv[None, :],
            "ident": ident, "zeros32": z32_h, "k1aug": k1g_h,
        })
    res = run_bass_kernel_spmd(nc, in_maps, list(range(B)))
    return np.stack([res.results[b]["out"] for b in range(B)], axis=0)


if __name__ == "__main__":
    rng = np.random.default_rng(0)
    s = 1.0 / np.sqrt(L)
    ins = {
        "x": rng.standard_normal((B, N, L)).astype(np.float32),
        "Wq": rng.standard_normal((H, L)).astype(np.float32) * s,
        "bq": rng.standard_normal((H,)).astype(np.float32) * s,
        "Wk": rng.standard_normal((H, L)).astype(np.float32) * s,
        "bk": rng.standard_normal((H,)).astype(np.float32) * s,
        "Wv": rng.standard_normal((L, L)).astype(np.float32) * s,
        "bv": rng.standard_normal((L,)).astype(np.float32) * s,
    }
    out = kernel(**ins)
    print("kernel ran, out shape", out.shape)


# revision 78
# speedup vs baseline: 1.0011x; 1.0011x over previous
"""Bag self-attention kernel for TRN2, data-parallel over the bag dim (8 cores).

Per core (one bag, x: [N=2048, L=1280], H=160):
  q = x@Wq.T + bq ; k = x@Wk.T (bk cancels) ; v = x@Wv.T
  S = q@k.T ; P = softmax(S) ; out = P@v + (x + bv)      (gamma = 1)

Mixed-precision split, driven by softmax sensitivity (logit noise at
near-tie rows is amplified by the value spread, so the q/k path needs
~FP22 while v and the attention weights tolerate fp8 pairs):

  - q/k projections and the energies S run in float32r (FP22 grade).
  - v projection and P@v run as fp8-e4m3 DoubleRow matmuls (2 k-tiles of
    128 per instruction, 0.5 PE cycles per output row). Operands are
    hi/lo fp8 splits (value = hi + lo); 3-term products
    A@B ~= Ah@Bh + Al@Bh + Ah@Bl give ~2^-9 relative error. Wv is
    pre-scaled by 64 on host so no fp8 entry is subnormal; the f32 PSUM
    result is descaled by 1/64 when re-quantized.
  - Softmax without transposes: pass-1 computes approximate S in [i,j]
    layout (single fp8 term from fp8 copies of q/k), DVE row-max gives
    m_i; the shift c_i = -(m_i - 1.5) is transposed into row form by
    tiny PE transposes and stored as an f32 augment row (partition 32)
    of the packed q1 tile, with a matching ones row in the k1 tile.
    Pass-2 computes S - c_i in [j,i] layout in f32r; ACT exp writes fp8
    E directly (E_top in ~[0.8, 25], inside e4m3's 240 max). Z comes
    from a ones-column of v; out = (E@v)/Z + (x + bv).
  - P@v is 2-term (Eh@vh + Eh@vl): v is kept exact via its fp8 pair, E
    carries the e4m3 quantization noise (renormalized by Z from the same
    quantized E, so one-hot rows cancel exactly).

Scheduling: the q/k projection stream (f32 x, DMA-heavy, PE-light) is
merged with the v-projection stream (fp8, PE-heavy) so the serialized
DMA pipe and the PE stay busy simultaneously. All input loads ride the
ACT HWDGE queue with posts staggered through the instruction stream
(pipe executes transfers in post order); output stores alternate the
SP/ACT queues. Biases are applied as per-partition ACT bias on the
PSUM->SBUF copies instead of extra bias matmul passes.
"""

import contextlib

import numpy as np
import ml_dtypes

import concourse.mybir as mybir
import concourse.tile as tile
from concourse import bacc
from concourse.bass_utils import run_bass_kernel_spmd

B, N, L, H = 8, 2048, 1280, 160
f32 = mybir.dt.float32
f32r = mybir.dt.float32r
bf16 = mybir.dt.bfloat16
fp8 = mybir.dt.float8e4
FP8 = ml_dtypes.float8_e4m3
DR = mybir.MatmulPerfMode.DoubleRow
Exp = mybir.ActivationFunctionType.Exp
Copy = mybir.ActivationFunctionType.Copy
Ident = mybir.ActivationFunctionType.Identity
ADD = mybir.AluOpType.add
SUB = mybir.AluOpType.subtract
MULT = mybir.AluOpType.mult
MAX = mybir.AluOpType.max

NL = L // 128          # 10 contraction k-tiles
NP = NL // 2           # 5 DoubleRow pairs
H0, H1 = 128, H - 128  # q/k head split 128 + 32
WS = 64.0              # host Wv scale (keeps fp8 Wv out of subnormals)
CM = 1.5               # row-max shift margin
NJ = N // 128          # 16 token chunks
NI4 = N // 512         # 4 i-macro chunks
MCH = [(1024, 1282), (0, 512), (512, 1024)]   # P@v m-chunks, Z-chunk first
ZC = 1280              # ones column (Z) position in v
VW = 1312              # v tile free width


def _build():
    nc = bacc.Bacc()
    dp = nc.declare_dram_parameter
    xf_d = dp("xf", [128, NL * N], f32r, isOutput=False)      # f32 xT [p,c,n]
    xh_d = dp("xh", [128, NL * N], fp8, isOutput=False)
    xl_d = dp("xl", [128, NL * N], fp8, isOutput=False)
    wq_d = dp("wq", [128, NL * H0], f32r, isOutput=False)
    wk_d = dp("wk", [128, NL * H0], f32r, isOutput=False)
    w1_d = dp("w1", [128, NL * 2 * H1], f32r, isOutput=False)  # q1|k1 packed
    wvh_d = dp("wvh", [128, NL * L], fp8, isOutput=False)
    wvl_d = dp("wvl", [128, NL * L], fp8, isOutput=False)
    bq0_d = dp("bq0", [128, 1], f32, isOutput=False)
    b1_d = dp("b1", [H1, 1], f32, isOutput=False)
    xr_d = dp("xresid", [N, L], f32, isOutput=False)
    id_d = dp("ident", [128, 128], f32, isOutput=False)
    z32_d = dp("zeros32", [32, N], f32r, isOutput=False)
    k1g_d = dp("k1aug", [32, N], f32r, isOutput=False)
    out_d = dp("out", [N, L], f32, isOutput=True)

    with tile.TileContext(nc) as tc:
        with (
            tc.tile_pool(name="const", bufs=1) as constp,
            tc.tile_pool(name="qk", bufs=1) as qkp,
            tc.tile_pool(name="vt", bufs=1) as vtp,
        ):
            es5 = contextlib.ExitStack()
            ep = es5.enter_context(tc.tile_pool(name="ep", bufs=2))
            es = contextlib.ExitStack()
            xtp = es.enter_context(tc.tile_pool(name="xt", bufs=1))
            wvp = es.enter_context(tc.tile_pool(name="wv", bufs=1, side="right"))
            wp = es.enter_context(tc.tile_pool(name="wp", bufs=1, side="right"))
            # ---- resident tiles (xt/wv/wp pools close before phase 5)
            xh = [xtp.tile([128, NL, 512], fp8, tag=f"xh{g}", name=f"xh{g}")
                  for g in range(4)]
            xl = [xtp.tile([128, NL, 512], fp8, tag=f"xl{g}", name=f"xl{g}")
                  for g in range(4)]
            wvh = [wvp.tile([128, NL, 512], fp8, tag="wvh", bufs=3,
                            name=f"wvh{mc}") for mc in range(3)]
            wvl = [wvp.tile([128, NL, 512], fp8, tag="wvl", bufs=3,
                            name=f"wvl{mc}") for mc in range(3)]
            wq = wp.tile([128, NL, H0], f32r, tag="wq")
            wka = wp.tile([128, 5, H0], f32r, tag="wka")
            wkb = wp.tile([128, 5, H0], f32r, tag="wkb")
            w1 = wp.tile([128, NL, 2 * H1], f32r, tag="w1")
            bq0 = wp.tile([128, 1], f32, tag="bq0")
            b1 = wp.tile([H1, 1], f32, tag="b1")
            ident = constp.tile([128, 128], f32, tag="ident")
            # f32r q/k: q0/k0 [128, N]; packed 32-head chunk + augments in
            # [64, N] tiles (q1a: rows 0..31 = q1, row 32 = -c_i, 33.. = 0;
            # k1a: rows 0..31 = k1, row 32 = ones, 33.. = 0)
            q0f = qkp.tile([128, N], f32r, tag="q0f")
            k0f = qkp.tile([128, N], f32r, tag="k0f")
            q1a = qkp.tile([64, N], f32r, tag="q1a")
            k1a = qkp.tile([64, N], f32r, tag="k1a")
            # fp8 copies of q/k for the pass-1 max estimate
            qh = qkp.tile([128, 2, N], fp8, tag="qh")
            kh = qkp.tile([128, 2, N], fp8, tag="kh")
            vh = [vtp.tile([128, 2, VW], fp8, tag=f"vh{jp}", name=f"vh{jp}")
                  for jp in range(8)]
            vl = [vtp.tile([128, 2, VW], fp8, tag=f"vl{jp}", name=f"vl{jp}")
                  for jp in range(8)]
            mall = constp.tile([128, 16], f32, tag="mall")
            mall4 = constp.tile([128, 64], f32, tag="mall4")

            xf_r = xf_d.rearrange("p (c n) -> p c n", c=NL)
            xh_r = xh_d.rearrange("p (c n) -> p c n", c=NL)
            xl_r = xl_d.rearrange("p (c n) -> p c n", c=NL)
            wvh_r = wvh_d.rearrange("p (c m) -> p c m", c=NL)
            wvl_r = wvl_d.rearrange("p (c m) -> p c m", c=NL)

            # memsets on Pool (no PE dependency)
            for t in (qh, kh):
                for p0 in (32, 64, 96):
                    nc.gpsimd.memset(t[p0:p0 + 32, 1, :], 0.0)
            for jp in range(8):
                nc.gpsimd.memset(vh[jp][:, :, ZC:VW], 0.0)
                nc.gpsimd.memset(vl[jp][:, :, ZC:VW], 0.0)
                nc.gpsimd.memset(vh[jp][:, :, ZC:ZC + 1], 1.0)

            # ---- fp8 x / Wv splits ride the ACT queue
            def xg_load(g, q=nc.scalar):
                csl = slice(g * 512, (g + 1) * 512)
                q.dma_start(out=xh[g], in_=xh_r[:, :, csl])
                q.dma_start(out=xl[g], in_=xl_r[:, :, csl])

            def wv_load(mc, q=nc.scalar):
                mlo2 = mc * 512
                mhi2 = min(mlo2 + 512, L)
                q.dma_start(out=wvh[mc][:, :, 0:mhi2 - mlo2],
                            in_=wvh_r[:, :, mlo2:mhi2])
                q.dma_start(out=wvl[mc][:, :, 0:mhi2 - mlo2],
                            in_=wvl_r[:, :, mlo2:mhi2])


            def acc3(ps, stat_h, stat_l, mov_h, mov_l):
                """fp8 DoubleRow 3-term product into ps."""
                for t in range(NP):
                    nc.tensor.matmul(ps, stat_h[:, 2 * t:2 * t + 2, :],
                                     mov_h[:, 2 * t:2 * t + 2, :],
                                     start=(t == 0), stop=False, perf_mode=DR)
                for t in range(NP):
                    nc.tensor.matmul(ps, stat_h[:, 2 * t:2 * t + 2, :],
                                     mov_l[:, 2 * t:2 * t + 2, :],
                                     start=False, stop=False, perf_mode=DR)
                for t in range(NP):
                    nc.tensor.matmul(ps, stat_l[:, 2 * t:2 * t + 2, :],
                                     mov_h[:, 2 * t:2 * t + 2, :],
                                     start=False, stop=(t == NP - 1),
                                     perf_mode=DR)

            ph5 = {}

            def s2_unit(i4, j, eh):
                isl = slice(i4 * 512, (i4 + 1) * 512)
                jsl = slice(j * 128, (j + 1) * 128)
                jp, half = j // 2, j % 2
                ps = ph5["s2ps"].tile([128, 512], f32, tag="s2",
                                      name=f"s2_{i4}_{j}")
                nc.tensor.matmul(ps, k0f[:, jsl], q0f[:, isl],
                                 start=True, stop=False)
                nc.tensor.matmul(ps, k1a[:, jsl], q1a[:, isl],
                                 start=False, stop=True)
                nc.scalar.activation(eh[jp][:, half, :], ps, Exp)

            def mk_e(i4):
                return [ep.tile([128, 2, 512], fp8, tag=f"eh{jp}",
                                name=f"eh{i4}_{jp}") for jp in range(8)]

            # ---- phases 1-4 merged. H-units (f32r q/k projection over 256
            # token cols) stream f32 x on the SP queue; V-units (fp8 v-proj
            # for one (tok, mc) chunk) consume the ACT-queue fp8 stream; the
            # pass-1 S~ units + row-max reduces + c augment rows follow the
            # last H-unit, and the S2(i4=0) exps close the phase.
            with (
                tc.tile_pool(name="vps", bufs=3, space="PSUM") as vps,
                tc.tile_pool(name="v32p", bufs=2) as v32p,
            ):
                def v_unit(tok, mc, dve):
                    g, gt = tok // 4, tok % 4
                    jsl = slice(gt * 128, (gt + 1) * 128)
                    jp, half = tok // 2, tok % 2
                    mlo, mhi = ((0, 512), (512, 1024), (1024, 1280))[mc]
                    ps = vps.tile([128, 512], f32, tag="v",
                                  name=f"v{tok}_{mc}")
                    psv = ps[:, 0:mhi - mlo]
                    acc3(psv, xh[g][:, :, jsl], xl[g][:, :, jsl],
                         wvh[mc][:, :, 0:mhi - mlo], wvl[mc][:, :, 0:mhi - mlo])
                    nc.scalar.activation(vh[jp][:, half, mlo:mhi], psv,
                                         Copy, scale=1.0 / WS)
                    if dve:
                        nc.vector.scalar_tensor_tensor(
                            out=vl[jp][:, half, mlo:mhi], in0=psv,
                            scalar=1.0 / WS, in1=vh[jp][:, half, mlo:mhi],
                            op0=MULT, op1=SUB)
                    else:
                        v32 = v32p.tile([128, 512], f32, tag="v32",
                                        name=f"v32_{tok}_{mc}")
                        nc.scalar.activation(v32[:, 0:mhi - mlo], psv, Copy,
                                             scale=1.0 / WS)
                        nc.gpsimd.tensor_sub(vl[jp][:, half, mlo:mhi],
                                             v32[:, 0:mhi - mlo],
                                             vh[jp][:, half, mlo:mhi])

                with (
                    tc.tile_pool(name="qkps", bufs=1, space="PSUM") as qkps,
                    tc.tile_pool(name="xfp", bufs=2) as xfp,
                ):
                    xfs = {}

                    def xf_load(hg):
                        # two c-halves: the consuming H-units' first five
                        # matmuls (k-tiles 0-4) unlock on the first half
                        if hg < 8:
                            xfs[hg] = xfp.tile([128, NL, 256], f32r, tag="xf",
                                               name=f"xf{hg}")
                            csl = slice(hg * 256, (hg + 1) * 256)
                            nc.scalar.dma_start(out=xfs[hg][:, 0:5, :],
                                                in_=xf_r[:, 0:5, csl])
                            nc.scalar.dma_start(out=xfs[hg][:, 5:10, :],
                                                in_=xf_r[:, 5:10, csl])

                    # ---- phase-1-4 input loads ride the single ACT queue.
                    # The serial DMA pipe runs transfers in post order; posts
                    # are staggered through the ACT instruction stream so the
                    # pipe order tracks the need order without conservative
                    # sem waits on large post batches.
                    nc.scalar.dma_start(out=wka[:, 0:3, :],
                                        in_=wk_d[:, 0:3 * H0])
                    xfs[0] = xfp.tile([128, NL, 256], f32r, tag="xf",
                                      name="xf0")
                    nc.scalar.dma_start(out=xfs[0][:, 0:3, :],
                                        in_=xf_r[:, 0:3, 0:256])
                    nc.scalar.dma_start(out=wka[:, 3:5, :],
                                        in_=wk_d[:, 3 * H0:5 * H0])
                    nc.scalar.dma_start(out=xfs[0][:, 3:6, :],
                                        in_=xf_r[:, 3:6, 0:256])
                    nc.scalar.dma_start(out=wkb, in_=wk_d[:, 5 * H0:NL * H0])
                    nc.scalar.dma_start(out=xfs[0][:, 6:10, :],
                                        in_=xf_r[:, 6:10, 0:256])
                    nc.scalar.dma_start(out=wq[:, 0:5, :],
                                        in_=wq_d[:, 0:5 * H0])
                    nc.scalar.dma_start(out=wq[:, 5:10, :],
                                        in_=wq_d[:, 5 * H0:NL * H0])
                    xf_load(1)
                    nc.scalar.dma_start(out=w1[:, 0:5, :],
                                        in_=w1_d[:, 0:5 * 2 * H1])
                    nc.scalar.dma_start(out=w1[:, 5:10, :],
                                        in_=w1_d[:, 5 * 2 * H1:NL * 2 * H1])
                    nc.scalar.dma_start(out=bq0, in_=bq0_d[:, :])
                    nc.scalar.dma_start(out=b1, in_=b1_d[:, :])

                    def hk_unit(hg):
                        isl = slice(hg * 256, (hg + 1) * 256)
                        xf = xfs[hg]
                        wkc = lambda c: (wka[:, c, :] if c < 5
                                         else wkb[:, c - 5, :])
                        ps_k = qkps.tile([H0, 256], f32, tag="k", name=f"k{hg}")
                        for c in range(NL):
                            nc.tensor.matmul(ps_k, wkc(c), xf[:, c, :],
                                             start=(c == 0), stop=(c == NL - 1))
                        nc.scalar.activation(k0f[:, isl], ps_k, Copy)
                        nc.scalar.activation(kh[:, 0, isl], ps_k, Copy)

                    def hq_unit(hg):
                        isl = slice(hg * 256, (hg + 1) * 256)
                        xf = xfs[hg]
                        ps_q = qkps.tile([H0, 256], f32, tag="q", name=f"q{hg}")
                        for c in range(NL):
                            nc.tensor.matmul(ps_q, wq[:, c, :], xf[:, c, :],
                                             start=(c == 0), stop=(c == NL - 1))
                        nc.scalar.activation(q0f[:, isl], ps_q, Ident,
                                             bias=bq0)
                        nc.scalar.activation(qh[:, 0, isl], ps_q, Ident,
                                             bias=bq0)

                    def h1_unit(hg):
                        isl = slice(hg * 256, (hg + 1) * 256)
                        xf = xfs.pop(hg)
                        ps_1 = qkps.tile([2 * H1, 256], f32, tag="qk1",
                                         name=f"qk1{hg}")
                        for c in range(NL):
                            nc.tensor.matmul(ps_1, w1[:, c, :], xf[:, c, :],
                                             start=(c == 0), stop=(c == NL - 1))
                        nc.scalar.activation(q1a[0:32, isl], ps_1[0:H1, :],
                                             Ident, bias=b1)
                        nc.scalar.activation(qh[0:H1, 1, isl], ps_1[0:H1, :],
                                             Ident, bias=b1)
                        nc.scalar.activation(k1a[0:32, isl], ps_1[H1:2 * H1, :],
                                             Copy)
                        nc.scalar.activation(kh[0:H1, 1, isl],
                                             ps_1[H1:2 * H1, :], Copy)
                        xf_load(hg + 2)

                    def h_trio(hg):
                        hk_unit(hg)
                        hq_unit(hg)
                        h1_unit(hg)

                    # g0 loads interleaved by first-use inside acc3: the
                    # hi@hi term needs only wvh0+xh0, hi@lo adds wvl0, and
                    # lo@hi adds xl0 last
                    nc.scalar.dma_start(out=wvh[0], in_=wvh_r[:, :, 0:512])
                    nc.scalar.dma_start(out=xh[0], in_=xh_r[:, :, 0:512])
                    nc.scalar.dma_start(out=wvl[0], in_=wvl_r[:, :, 0:512])
                    nc.scalar.dma_start(out=xl[0], in_=xl_r[:, :, 0:512])
                    h_trio(0)
                    h_trio(1)
                    v_unit(0, 0, dve=True)
                    v_unit(1, 0, dve=True)
                    h_trio(2)
                    wv_load(1)
                    v_unit(2, 0, dve=True)
                    v_unit(3, 0, dve=True)
                    h_trio(3)
                    xg_load(1)
                    v_unit(0, 1, dve=True)
                    v_unit(1, 1, dve=True)
                    h_trio(4)
                    wv_load(2)
                    v_unit(2, 1, dve=True)
                    v_unit(3, 1, dve=True)
                    h_trio(5)
                    xg_load(2)
                    v_unit(4, 0, dve=True)
                    v_unit(5, 0, dve=True)
                    h_trio(6)
                    xg_load(3)
                    v_unit(6, 0, dve=True)
                    v_unit(7, 0, dve=True)
                    h_trio(7)
                    nc.scalar.dma_start(out=ident, in_=id_d[:, :])
                    nc.scalar.dma_start(out=q1a[32:64, :], in_=z32_d[:, :])
                    nc.scalar.dma_start(out=k1a[32:64, :], in_=k1g_d[:, :])
                    v_unit(4, 1, dve=True)
                    v_unit(5, 1, dve=True)

                s1_long = ([(t, 0) for t in range(8, 16)]
                           + [(t, 1) for t in range(8, 16)])
                s1_short = [(t, 2) for t in range(16)]
                s1_short_at = {ic: 1 for ic in range(16)}
                s1_short_at[0] = 2
                s1_short_at[1] = 2
                s1_short_at[14] = 0
                s1_short_at[15] = 0
                tail_v = [(t, 1) for t in (6, 7)]

                cstack = contextlib.ExitStack()
                cps = cstack.enter_context(
                    tc.tile_pool(name="cps", bufs=1, space="PSUM"))
                with tc.tile_pool(name="s1ps", bufs=2, space="PSUM") as s1ps:
                    def c_rows(ic):
                        isl = slice(ic * 128, (ic + 1) * 128)
                        pt = cps.tile([1, 128], f32, tag="ct", name=f"ct{ic}")
                        nc.tensor.matmul(pt, mall[:, ic:ic + 1], ident,
                                         is_transpose=True)
                        nc.scalar.activation(q1a[32:33, isl], pt, Copy, bias=CM)

                    # pass-1 S~ + row-max: per ic, two [128, 1024] PSUM
                    # halves (2 DR matmuls + one DVE reduce each, chunk maxes
                    # into mall4, second-stage max into mall). bufs=2 means a
                    # half's buffer is recycled only after its reduce from the
                    # previous ic, which completed long ago, so the PE never
                    # waits on the DVE chain. V-units (gpsimd vl path, keeping
                    # the DVE clear for reduces) fill the PE.
                    ri = 0
                    for ic in range(16):
                        for jh in range(2):
                            s1t = s1ps.tile([128, 1024], f32, tag="s1",
                                            name=f"s1_{ic}_{jh}")
                            for jc in (2 * jh, 2 * jh + 1):
                                nc.tensor.matmul(
                                    s1t[:, (jc % 2) * 512:(jc % 2) * 512 + 512],
                                    qh[:, :, ic * 128:(ic + 1) * 128],
                                    kh[:, :, jc * 512:(jc + 1) * 512],
                                    start=True, stop=True, perf_mode=DR)
                            nc.vector.tensor_reduce(
                                mall4[:, 2 * ic + jh:2 * ic + jh + 1], s1t,
                                axis=mybir.AxisListType.X, op=MAX)
                        nc.vector.tensor_reduce(
                            mall[:, ic:ic + 1], mall4[:, 2 * ic:2 * ic + 2],
                            axis=mybir.AxisListType.X, op=MAX, negate=True)
                        v_unit(*s1_long[ic], dve=False)
                        for _ in range(s1_short_at.get(ic, 0)):
                            v_unit(*s1_short[ri], dve=False)
                            ri += 1
                        if ic >= 2:
                            c_rows(ic - 2)

                # s1ps/cps banks are free now for the i4=0 S2 tail
                # tail: remaining v units + late c rows + the S2(i4=0) units
                # (s2(0,*) reads only q1a cols 0:512, i.e. c0-3, so c14/c15
                # can ride along here without stalling the PE). Shares the
                # phase-5 s2 PSUM pool so there is no pool-close barrier at
                # the phase boundary.
                e_cur = mk_e(0)
                s2ps = es5.enter_context(
                    tc.tile_pool(name="s2ps", bufs=4, space="PSUM",
                                 side="right"))
                ph5["s2ps"] = s2ps
                ti, s2j = 0, 0
                while ti < len(tail_v) or s2j < NJ:
                    if ti < len(tail_v):
                        v_unit(*tail_v[ti], dve=True)
                        ti += 1
                    if s2j == 2:
                        c_rows(14)
                    elif s2j == 4:
                        c_rows(15)
                    if s2j < NJ:
                        s2_unit(0, s2j, e_cur)
                        s2j += 1
                cstack.close()

            es.close()   # free x / wv / weight SBUF before attention
            # ---- phase 5: attention; S2(i4) interleaved with P@v(i4-1)
            with (
                tc.tile_pool(name="ops", bufs=1, space="PSUM") as ops,
                tc.tile_pool(name="stg", bufs=2) as stg,
            ):
                def pv_unit(i4, isub, mc, pso, eh, split_out=False,
                            mch=None, st_q=None):
                    i0 = i4 * 512 + isub * 128
                    esl = slice(isub * 128, (isub + 1) * 128)
                    mlo, mhi = mch if mch is not None else MCH[mc]
                    ps = ops.tile([128, 512], f32, tag=f"o{mc}",
                                  bufs=(2 if mc == 2 else 1),
                                  name=f"o{i4}_{isub}_{mc}_{mlo}")
                    ps = ps[:, 0:mhi - mlo]
                    for jp in range(8):
                        nc.tensor.matmul(
                            ps, eh[jp][:, :, esl], vh[jp][:, :, mlo:mhi],
                            start=(jp == 0), stop=False, perf_mode=DR)
                    for jp in range(8):
                        nc.tensor.matmul(
                            ps, eh[jp][:, :, esl], vl[jp][:, :, mlo:mhi],
                            start=False, stop=(jp == 7), perf_mode=DR)
                    if mc == 0:
                        recip = stg.tile([128, 1], f32, tag="recip",
                                         name=f"recip{i4}_{isub}")
                        pso["recip"] = recip
                        nc.vector.reciprocal(recip, ps[:, 256:257])
                        xr = stg.tile([128, L], f32, tag="xr",
                                      name=f"xr{i4}_{isub}")
                        pso["xr"] = xr
                        nc.sync.dma_start(out=xr, in_=xr_d[i0:i0 + 128, :])
                    mwid = min(mhi, L) - mlo
                    ot = stg.tile([128, 512], f32, tag=f"ot{mc}",
                                  name=f"ot{i4}_{isub}_{mc}_{mlo}")
                    halves = ((0, mwid // 2), (mwid // 2, mwid)) if split_out \
                        else ((0, mwid),)
                    for nh, (lo, hi) in enumerate(halves):
                        nc.vector.scalar_tensor_tensor(
                            out=ot[:, lo:hi], in0=ps[:, lo:hi],
                            scalar=pso["recip"],
                            in1=pso["xr"][:, mlo + lo:mlo + hi],
                            op0=MULT, op1=ADD)
                        q = st_q if st_q is not None else nc.sync
                        q.dma_start(
                            out=out_d[i0:i0 + 128, mlo + lo:mlo + hi],
                            in_=ot[:, lo:hi])

                for i4 in range(1, NI4 + 1):
                    e_prev, pso_prev = e_cur, {}
                    pv_units = [(isub, mc) for isub in range(4)
                                for mc in range(3)]
                    if i4 <= NI4 - 1:
                        e_cur = mk_e(i4)
                        if i4 == 1:
                            pv_at = {5: (0, 3), 9: (3, 6), 12: (6, 9),
                                     15: (9, 12)}
                        else:
                            pv_at = {j: (j - 2, j - 1) for j in range(2, 14)}
                        for j in range(NJ):
                            s2_unit(i4, j, e_cur)
                            if j in pv_at:
                                lo, hi = pv_at[j]
                                for isub, mc in pv_units[lo:hi]:
                                    pv_unit(i4 - 1, isub, mc, pso_prev,
                                            e_prev)
                    else:
                        for un, (isub, mc) in enumerate(pv_units[:-1]):
                            pv_unit(i4 - 1, isub, mc, pso_prev, e_prev,
                                    split_out=(un >= 9),
                                    st_q=(nc.scalar if un % 2 else nc.sync))
                        # last unit (isub 3, mc 2) in two m-halves so the
                        # first half's store chain overlaps the second
                        # half's matmuls
                        mlo, mhi = MCH[2]
                        mmid = (mlo + mhi) // 2
                        pv_unit(i4 - 1, 3, 2, pso_prev, e_prev,
                                mch=(mlo, mmid), st_q=nc.scalar)
                        pv_unit(i4 - 1, 3, 2, pso_prev, e_prev,
                                mch=(mmid, mhi), st_q=nc.sync)
            es5.close()

    nc.finalize()
    return nc


_NC = None


def _get_nc():
    global _NC
    if _NC is None:
        _NC = _build()
    return _NC


def _split8(a):
    hi = a.astype(FP8)
    lo = (a - hi.astype(np.float32)).astype(FP8)
    return hi, lo


def _wpackf(WT):
    """WT: [L, h] f32. Returns [128, NL*h] f32 with k-tile layout
    [p, c, h]."""
    Lh, h = WT.shape
    full = WT.reshape(NL, 128, h).transpose(1, 0, 2)
    return np.ascontiguousarray(full.reshape(128, NL * h))


def kernel(x, Wq, bq, Wk, bk, Wv, bv):
    x = np.asarray(x, np.float32)
    Wq = np.asarray(Wq, np.float32); bq = np.asarray(bq, np.float32)
    Wk = np.asarray(Wk, np.float32)
    Wv = np.asarray(Wv, np.float32); bv = np.asarray(bv, np.float32)

    WqT = Wq.T                    # [L, H]
    WkT = Wk.T
    wqf = _wpackf(np.ascontiguousarray(WqT[:, :H0]))
    wkf = _wpackf(np.ascontiguousarray(WkT[:, :H0]))
    w1c = np.concatenate([WqT[:, H0:], WkT[:, H0:]], axis=1)  # [L, 64]
    w1f = _wpackf(np.ascontiguousarray(w1c))
    WvTs = Wv.T * WS
    wvh_, wvl_ = _split8(
        WvTs.reshape(NL, 128, L).transpose(1, 0, 2))
    wvh = np.ascontiguousarray(wvh_.reshape(128, NL * L))
    wvl = np.ascontiguousarray(wvl_.reshape(128, NL * L))

    nc = _get_nc()
    ident = np.eye(128, dtype=np.float32)
    z32_h = np.zeros((32, N), np.float32)
    k1g_h = np.zeros((32, N), np.float32)
    k1g_h[0, :] = 1.0
    bq0_h = np.ascontiguousarray(bq[:H0, None])
    b1_h = np.ascontiguousarray(bq[H0:, None])
    in_maps = []
    for b in range(B):
        xT3 = np.ascontiguousarray(x[b].T).reshape(NL, 128, N).transpose(1, 0, 2)
        xh, xl = _split8(xT3)
        in_maps.append({
            "xf": np.ascontiguousarray(xT3.reshape(128, NL * N)),
            "xh": np.ascontiguousarray(xh.reshape(128, NL * N)),
            "xl": np.ascontiguousarray(xl.reshape(128, NL * N)),
            "wq": wqf, "wk": wkf, "w1": w1f, "wvh": wvh, "wvl": wvl,
            "bq0": bq0_h, "b1": b1_h,
            "xresid": x[b] + bv[None, :],
            "ident": ident, "zeros32": z32_h, "k1aug": k1g_h,
        })
    res = run_bass_kernel_spmd(nc, in_maps, list(range(B)))
    return np.stack([res.results[b]["out"] for b in range(B)], axis=0)


if __name__ == "__main__":
    rng = np.random.default_rng(0)
    s = 1.0 / np.sqrt(L)
    ins = {
        "x": rng.standard_normal((B, N, L)).astype(np.float32),
        "Wq": rng.standard_normal((H, L)).astype(np.float32) * s,
        "bq": rng.standard_normal((H,)).astype(np.float32) * s,
        "Wk": rng.standard_normal((H, L)).astype(np.float32) * s,
        "bk": rng.standard_normal((H,)).astype(np.float32) * s,
        "Wv": rng.standard_normal((L, L)).astype(np.float32) * s,
        "bv": rng.standard_normal((L,)).astype(np.float32) * s,
    }
    out = kernel(**ins)
    print("kernel ran, out shape", out.shape)
